# revision 15
# baseline (speedup 1.0000x reference)
"""Trainium2 Bass kernel: CausalSelfAttention with FIRE additive mask.

Computes, for x:[4,2048,768], mask:[1,12,2048,2048]:
    qkv = x @ W_attn + b_attn ; q,k,v = split(qkv)
    y   = softmax(q k^T / 8 + mask) v   (per batch/head)
    out = y @ W_proj + b_proj

Sharding: tokens are split into 8 blocks of 256 along T (all batches, all
heads on every core). Each core computes q/k/v for its own 1024 (b,t) rows,
AllGathers k and v (bf16) across the 8 cores, then runs attention + output
projection for its own 256 query rows of every (batch, head). The v
all-gather is issued right after the v projection so it overlaps the qk
projection phase.

Softmax uses exp(s + m) = exp(s) * exp(m): the host feeds exp(mask) (bf16),
so the causal -1e9 entries become exact zeros and the device does one Exp
pass (ScalarE) and one elementwise multiply (VectorE, bf16 2x mode) per
score tile. Softmax denominators come free from the attention matmul via a
ones-column appended to v; normalization happens on the [65,256] y^T tile.
The two heads of a pair run their K=64 score matmuls in disjoint PE
row-groups (base partitions 0/64) so they execute concurrently.

All big operands stay transposed ([feature, token]) so every matmul's
contraction dim sits on SBUF partitions without any on-device transposes.
"""
from contextlib import ExitStack

import numpy as np
import ml_dtypes

import concourse.bass as bass
from concourse import bacc
import concourse.mybir as mybir
import concourse.tile as tile
from concourse.bass_utils import run_bass_kernel_spmd

BF16 = mybir.dt.bfloat16
F32 = mybir.dt.float32
AF = mybir.ActivationFunctionType
ALU = mybir.AluOpType

NCORES = 8
B, T, C, H, HD = 4, 2048, 768, 12, 64
TQ = T // NCORES          # 256 query rows per core per (b,h)
TOK = B * TQ              # 1024 (b,t) token rows owned per core
CK = C // 128             # 6 contraction chunks for the projections
KCH = T // 128            # 16 key chunks of 128
KV_PART = C * TOK         # elements of one kv half (k^T or v), = 786432

NP_BF16 = ml_dtypes.bfloat16


def build_kernel(debug: bool = False, fake_ag: bool = False,
                 repeat: int = 1) -> bass.Bass:
    assert not (debug and repeat > 1)
    nc = bacc.Bacc(None, num_devices=NCORES)

    xt = nc.dram_tensor("xt", [C, TOK], BF16, kind="ExternalInput")
    wqk = nc.dram_tensor("wqk", [C, 2 * C], BF16, kind="ExternalInput")
    wv = nc.dram_tensor("wv", [C, C], BF16, kind="ExternalInput")
    wp = nc.dram_tensor("wp", [C, C], BF16, kind="ExternalInput")
    bq8 = nc.dram_tensor("bq8", [C], F32, kind="ExternalInput")
    bk = nc.dram_tensor("bk", [C], F32, kind="ExternalInput")
    bvv = nc.dram_tensor("bvv", [C], F32, kind="ExternalInput")
    bpp = nc.dram_tensor("bpp", [C], F32, kind="ExternalInput")
    em = nc.dram_tensor("em", [H, KCH, 128, TQ], BF16, kind="ExternalInput")
    out_d = nc.dram_tensor("out", [TOK, C], F32, kind="ExternalOutput")
    if debug:
        dbg_qt = nc.dram_tensor("dbg_qt", [128, CK, TOK], BF16, kind="ExternalOutput")
        dbg_kv = nc.dram_tensor("dbg_kv", [2, KV_PART], BF16, kind="ExternalOutput")
        dbg_ag = nc.dram_tensor("dbg_ag", [NCORES, 2, KV_PART], BF16, kind="ExternalOutput")
        dbg_es = nc.dram_tensor("dbg_es", [128, KCH * TQ], BF16, kind="ExternalOutput")
        dbg_p = nc.dram_tensor("dbg_p", [128, KCH * TQ], BF16, kind="ExternalOutput")

    with tile.TileContext(nc) as tc, ExitStack() as ctx:
        consts = ctx.enter_context(tc.tile_pool(name="consts", bufs=1))
        persist = ctx.enter_context(tc.tile_pool(name="persist", bufs=1))
        dram = ctx.enter_context(tc.tile_pool(name="dram", bufs=1, space="DRAM"))

        # ---- constant loads ------------------------------------------------
        xt_sb = consts.tile([128, CK, TOK], BF16)
        nc.sync.dma_start(xt_sb[:], xt.rearrange("(ck p) t -> p ck t", p=128))
        wqk_sb = consts.tile([128, CK, 2 * C], BF16)
        nc.sync.dma_start(wqk_sb[:], wqk.rearrange("(ck p) n -> p ck n", p=128))
        wv_sb = consts.tile([128, CK, C], BF16)
        nc.sync.dma_start(wv_sb[:], wv.rearrange("(ck p) n -> p ck n", p=128))
        wp_sb = consts.tile([128, CK, C], BF16)
        nc.sync.dma_start(wp_sb[:], wp.rearrange("(ck p) o -> p ck o", p=128))
        bq8_sb = consts.tile([128, CK], F32)
        nc.sync.dma_start(bq8_sb[:], bq8.rearrange("(j p) -> p j", p=128))
        bk_sb = consts.tile([128, CK], F32)
        nc.sync.dma_start(bk_sb[:], bk.rearrange("(j p) -> p j", p=128))
        bv_bc = consts.tile([128, C], F32)
        nc.sync.dma_start(bv_bc[:], bass.AP(bvv, 0, [[0, 128], [1, C]]))
        bp_bc = consts.tile([128, C], F32)
        nc.sync.dma_start(bp_bc[:], bass.AP(bpp, 0, [[0, 128], [1, C]]))

        qt_sb = persist.tile([128, CK, TOK], BF16)  # q^T/8+bq/8, heads paired
        ycat = [persist.tile([128, CK, TQ], BF16, name=f"ycat{b}")
                for b in range(B)]  # head-PAIRED y^T: rows = (h%2)*64+hd

        k_own = dram.tile([KV_PART], BF16)
        v_own = dram.tile([KV_PART], BF16)
        aspace = "Local" if fake_ag else "Shared"
        ag_ks = [dram.tile([NCORES, KV_PART], BF16, addr_space=aspace,
                           name=f"ag_k{r}") for r in range(repeat)]
        ag_vs = [dram.tile([NCORES, KV_PART], BF16, addr_space=aspace,
                           name=f"ag_v{r}") for r in range(repeat)]
        kt_view = k_own[:].rearrange("(c t) -> c t", c=C)
        v_view = v_own[:].rearrange("(h b q d) -> h b q d", h=H, b=B, q=TQ)
        rg = [list(range(NCORES))]

        for _rep in range(repeat):
            ag_k, ag_v = ag_ks[_rep], ag_vs[_rep]
            # ---- phase 1: v = x W_v + bv, stored [h, b, tq, hd] ------------
            with tc.tile_pool(name="p1ps", bufs=3, space="PSUM") as pps, \
                 tc.tile_pool(name="p1ev", bufs=3) as pev:
                for i in range(TOK // 128):
                    bb, tq0 = i // 2, (i % 2) * 128
                    for vh in range(2):
                        ps = pps.tile([128, 384], F32, tag="ps2")
                        for ck in range(CK):
                            nc.tensor.matmul(
                                ps[:],
                                xt_sb[:, ck, 128 * i:128 * (i + 1)],
                                wv_sb[:, ck, 384 * vh:384 * (vh + 1)],
                                start=(ck == 0), stop=(ck == CK - 1),
                            )
                        vt = pev.tile([128, 384], BF16, tag="vev")
                        nc.vector.scalar_tensor_tensor(
                            vt[:], ps[:], 1.0, bv_bc[:, 384 * vh:384 * (vh + 1)],
                            op0=ALU.mult, op1=ALU.add,
                        )
                        dst = v_view[6 * vh:6 * (vh + 1), bb, tq0:tq0 + 128, :]
                        nc.sync.dma_start(
                            dst.rearrange("h q d -> q h d"),
                            vt[:].rearrange("q (h d) -> q h d", d=HD),
                        )

                if fake_ag:
                    for r in range(NCORES):
                        nc.sync.dma_start(ag_v[r], v_own[:])
                else:
                    nc.gpsimd.collective_compute(
                        "AllGather", ALU.bypass, replica_groups=rg,
                        ins=[v_own[:]], outs=[ag_v[:]],
                    )

                # ---- phase 2: qk^T, evacuate with bias (q also * 1/8) ------
                for j in range(2 * CK):
                    for th in range(2):
                        ps = pps.tile([128, 512], F32, tag="ps1")
                        for ck in range(CK):
                            nc.tensor.matmul(
                                ps[:],
                                wqk_sb[:, ck, 128 * j:128 * (j + 1)],
                                xt_sb[:, ck, 512 * th:512 * (th + 1)],
                                start=(ck == 0), stop=(ck == CK - 1),
                            )
                        if j < CK:  # q columns
                            nc.scalar.activation(
                                qt_sb[:, j, 512 * th:512 * (th + 1)], ps[:],
                                AF.Identity, bias=bq8_sb[:, j:j + 1], scale=0.125,
                            )
                        else:  # k columns -> HBM for the all-gather
                            kt_t = pev.tile([128, 512], BF16, tag="ktev")
                            nc.scalar.activation(
                                kt_t[:], ps[:], AF.Identity,
                                bias=bk_sb[:, j - CK:j - CK + 1],
                            )
                            nc.sync.dma_start(
                                kt_view[128 * (j - CK):128 * (j - CK + 1),
                                        512 * th:512 * (th + 1)],
                                kt_t[:],
                            )

            if fake_ag:
                for r in range(NCORES):
                    nc.sync.dma_start(ag_k[r], k_own[:])
            else:
                nc.gpsimd.collective_compute(
                    "AllGather", ALU.bypass, replica_groups=rg,
                    ins=[k_own[:]], outs=[ag_k[:]],
                )

            # ---- phase 3: attention ----------------------------------------
            with tc.tile_pool(name="kt", bufs=2) as ktpool, \
                 tc.tile_pool(name="va", bufs=2) as vapool, \
                 tc.tile_pool(name="emp", bufs=2) as empool, \
                 tc.tile_pool(name="esp", bufs=2) as espool, \
                 tc.tile_pool(name="pp", bufs=2) as ppool, \
                 tc.tile_pool(name="sml", bufs=2) as smlpool, \
                 tc.tile_pool(name="sps", bufs=2, space="PSUM") as spool, \
                 tc.tile_pool(name="yts", bufs=2, space="PSUM") as ytpool:
                for hp in range(H // 2):
                    em_t = [empool.tile([128, KCH * TQ], BF16, name=f"em{hh}")
                            for hh in range(2)]
                    for hh in range(2):
                        nc.sync.dma_start(
                            em_t[hh].rearrange("p (kc q) -> p kc q", q=TQ),
                            em[2 * hp + hh].rearrange("kc p q -> p kc q"),
                        )
                    for b in range(B):
                        ktp = ktpool.tile([128, T], BF16)
                        for r in range(NCORES):
                            src = ag_k[r].rearrange("(c t) -> c t", c=C)
                            nc.sync.dma_start(
                                ktp[:, TQ * r:TQ * (r + 1)],
                                src[128 * hp:128 * (hp + 1), TQ * b:TQ * (b + 1)],
                            )
                        vas, ess = [], []
                        for hh in range(2):
                            h = 2 * hp + hh
                            va = vapool.tile([128, KCH, HD + 1], BF16,
                                             name=f"va{hh}")
                            nc.vector.memset(va[:, :, HD:HD + 1], 1.0)
                            for r in range(NCORES):
                                src = ag_v[r].rearrange(
                                    "(hx bx q d) -> hx bx q d", hx=H, bx=B, q=TQ)
                                nc.sync.dma_start(
                                    va[:, 2 * r:2 * r + 2, 0:HD],
                                    src[h, b].rearrange("(cc p) d -> p cc d", p=128),
                                )
                            vas.append(va)
                            ess.append(espool.tile([128, KCH * TQ], BF16,
                                                   name=f"es{hh}"))
                        # scores: head pair interleaved -> disjoint PE row-groups
                        for g in range(4):
                            sps = [spool.tile([128, 4 * TQ], F32, tag=f"sp{hh}",
                                              name=f"sp{hh}", bufs=1)
                                   for hh in range(2)]
                            for kk in range(4):
                                kc = 4 * g + kk
                                for hh in range(2):
                                    p0 = 64 * hh
                                    nc.tensor.matmul(
                                        sps[hh][:, TQ * kk:TQ * (kk + 1)],
                                        ktp[p0:p0 + 64, 128 * kc:128 * (kc + 1)],
                                        qt_sb[p0:p0 + 64, hp, TQ * b:TQ * (b + 1)],
                                        start=True, stop=True,
                                    )
                            for hh in range(2):
                                nc.scalar.activation(
                                    ess[hh][:, 4 * TQ * g:4 * TQ * (g + 1)],
                                    sps[hh][:], AF.Exp)
                        for hh in range(2):
                            h = 2 * hp + hh
                            va, es = vas[hh], ess[hh]
                            p_sb = ppool.tile([128, KCH * TQ], BF16)
                            half = KCH * TQ // 2
                            for m in range(2):
                                nc.vector.tensor_mul(
                                    p_sb[:, half * m:half * (m + 1)],
                                    es[:, half * m:half * (m + 1)],
                                    em_t[hh][:, half * m:half * (m + 1)],
                                )
                            yt = ytpool.tile([HD + 1, TQ], F32)
                            for kc in range(KCH):
                                nc.tensor.matmul(
                                    yt[:],
                                    va[:, kc, :],
                                    p_sb[:, TQ * kc:TQ * (kc + 1)],
                                    start=(kc == 0), stop=(kc == KCH - 1),
                                )
                            ssum = smlpool.tile([HD + 1, TQ], F32, tag="ssum")
                            nc.vector.reciprocal(
                                ssum[HD:HD + 1, :], yt[HD:HD + 1, :])
                            rsum0 = smlpool.tile([1, TQ], F32, tag="rsum0")
                            nc.sync.dma_start(rsum0[:], ssum[HD:HD + 1, :])
                            rbc = smlpool.tile([HD, TQ], F32, tag="rbc")
                            nc.gpsimd.partition_broadcast(rbc[:], rsum0[0:1, :])
                            # normalized y^T -> ycat; odd head rows go to
                            # partitions 64..127 via a small shifting DMA
                            if hh == 0:
                                nc.vector.tensor_mul(
                                    ycat[b][0:HD, hp, :], yt[0:HD, :], rbc[:])
                            else:
                                ysh = smlpool.tile([HD, TQ], BF16, tag="ysh")
                                nc.vector.tensor_mul(
                                    ysh[:], yt[0:HD, :], rbc[:])
                                nc.sync.dma_start(
                                    ycat[b][HD:128, hp, :], ysh[:])
                            if debug and hp == 0 and b == 0 and hh == 0:
                                nc.sync.dma_start(dbg_es[:], es[:])
                                nc.sync.dma_start(dbg_p[:], p_sb[:])

            if debug:
                nc.sync.dma_start(dbg_qt[:], qt_sb[:])
                nc.sync.dma_start(dbg_kv[0], k_own[:])
                nc.sync.dma_start(dbg_kv[1], v_own[:])
                for r in range(NCORES):
                    nc.sync.dma_start(dbg_ag[r, 0], ag_k[r])
                    nc.sync.dma_start(dbg_ag[r, 1], ag_v[r])

            # ---- phase 4: out = y_cat W_proj + b_proj (K=128 head pairs) ---
            with tc.tile_pool(name="p4ps", bufs=3, space="PSUM") as pps, \
                 tc.tile_pool(name="p4ev", bufs=3) as pev:
                for b in range(B):
                    for i in range(2):
                        for oh in range(2):
                            ps = pps.tile([128, 384], F32)
                            for ck in range(CK):
                                nc.tensor.matmul(
                                    ps[:],
                                    ycat[b][:, ck, 128 * i:128 * (i + 1)],
                                    wp_sb[:, ck, 384 * oh:384 * (oh + 1)],
                                    start=(ck == 0), stop=(ck == CK - 1),
                                )
                            ot = pev.tile([128, 384], F32)
                            nc.vector.scalar_tensor_tensor(
                                ot[:], ps[:], 1.0,
                                bp_bc[:, 384 * oh:384 * (oh + 1)],
                                op0=ALU.mult, op1=ALU.add,
                            )
                            nc.sync.dma_start(
                                out_d[TQ * b + 128 * i:TQ * b + 128 * (i + 1),
                                      384 * oh:384 * (oh + 1)],
                                ot[:],
                            )
    return nc


_CACHE: dict = {}


def _prep_shared(W_attn, b_attn, W_proj, b_proj, fire_causal_mask):
    wqk = np.ascontiguousarray(W_attn[:, :2 * C]).astype(NP_BF16)
    wv = np.ascontiguousarray(W_attn[:, 2 * C:]).astype(NP_BF16)
    wp = W_proj.astype(NP_BF16)
    bq8 = (b_attn[:C] / 8.0).astype(np.float32)
    bk = b_attn[C:2 * C].astype(np.float32)
    bvv = b_attn[2 * C:].astype(np.float32)
    bpp = b_proj.astype(np.float32)
    em_full = np.exp(fire_causal_mask[0].astype(np.float32))  # [H, T, T]
    return wqk, wv, wp, bq8, bk, bvv, bpp, em_full


def prepare_in_maps(x, fire_causal_mask, W_attn, b_attn, W_proj, b_proj):
    x = np.asarray(x, np.float32)
    fire_causal_mask = np.asarray(fire_causal_mask, np.float32)
    W_attn = np.asarray(W_attn, np.float32)
    b_attn = np.asarray(b_attn, np.float32)
    W_proj = np.asarray(W_proj, np.float32)
    b_proj = np.asarray(b_proj, np.float32)

    wqk, wv, wp, bq8, bk, bvv, bpp, em_full = _prep_shared(
        W_attn, b_attn, W_proj, b_proj, fire_causal_mask)

    in_maps = []
    for c in range(NCORES):
        sl = slice(TQ * c, TQ * (c + 1))
        xt_c = np.ascontiguousarray(
            x[:, sl, :].transpose(2, 0, 1).reshape(C, TOK)).astype(NP_BF16)
        # em packed transposed per key-chunk: [H, KCH, 128, TQ]
        em_c = np.ascontiguousarray(
            em_full[:, sl, :].reshape(H, TQ, KCH, 128).transpose(0, 2, 3, 1)
        ).astype(NP_BF16)
        in_maps.append({
            "xt": xt_c, "wqk": wqk, "wv": wv, "wp": wp,
            "bq8": bq8, "bk": bk, "bvv": bvv, "bpp": bpp, "em": em_c,
        })
    return in_maps


def run_spmd(in_maps, **kwargs):
    if "nc" not in _CACHE:
        nc = build_kernel()
        nc.finalize()
        _CACHE["nc"] = nc
    return run_bass_kernel_spmd(
        _CACHE["nc"], in_maps, list(range(NCORES)), **kwargs)


def assemble_output(results) -> np.ndarray:
    out = np.empty((B, T, C), np.float32)
    for c in range(NCORES):
        out[:, TQ * c:TQ * (c + 1), :] = results[c]["out"].reshape(B, TQ, C)
    return out


def kernel(x, fire_causal_mask, W_attn, b_attn, W_proj, b_proj):
    in_maps = prepare_in_maps(
        x, fire_causal_mask, W_attn, b_attn, W_proj, b_proj)
    res = run_spmd(in_maps)
    return assemble_output(res.results)
